# revision 9
# baseline (speedup 1.0000x reference)
"""TRN2 Bass kernel for GQA attention (nn_Attention_13030930776201).

Reference computation (B=2, T=S=1024, D=2048, 16 Q heads / 4 KV heads, H=128):
    q = Xq @ Wq; k = Xkv @ Wk; v = Xkv @ Wv         (DenseGeneral projections)
    q, k = RoPE(q, q_pos), RoPE(k, kv_pos)
    out = softmax(q k^T) v  @ Wo                     (GQA, scale=1.0, no mask)

Sharding: 8 cores = 2 (batch) x 4 (KV-head group). Each core computes one
batch's attention for one KV head + its 4 Q heads, producing a partial
(1024, 2048) output; the host sums the 4 partials per batch.

Per-core dataflow (all layouts chosen so NO on-device transposes of
activations are needed; host passes X pre-transposed):
    QT[h',t]  = Wq[d,h'].T @ XqT[d,t]      (f32r matmuls, fp32 PSUM)
    KT[h',s]  = Wk[d,h'].T @ XkvT[d,s]
    VT[h',s]  = Wv[d,h'].T @ XkvT[d,s], PE-transposed to V[s,h'] (bf16)
    RoPE on QT/KT in fp32 (DVE + SBUF-to-SBUF DMA half swap)
    S^T[s,t]  = KT[h',s].T @ QT[h',t]      (f32r) ; expS = exp(S^T) (ACT, bf16)
    sums[t]   = ones[s].T @ expS[s,t]      (PE column-sum trick)
    O^T[h',t] = V[s,h'].T @ expS[s,t]      (bf16), normalized by 1/sums -> fp16
    out[t,o]  = O^T[h',t].T @ Wo[h',o]     (fp16), written as fp16 partial

float32r gives tf32-like precision at full PE rate (measured rel err 1.5e-4
vs bf16's 2.4e-3), which keeps softmax logits accurate; the V/O path is
precision-tolerant so it runs in bf16/fp16.
"""

import sys

if "/opt/trn_rl_repo" not in sys.path:
    sys.path.insert(0, "/opt/trn_rl_repo")

from contextlib import ExitStack

import numpy as np

import concourse.bass as bass
import concourse.tile as tile
from concourse import bacc, mybir
from concourse.bass_utils import run_bass_kernel_spmd
from concourse.masks import make_identity

P = 128          # partitions / head dim
T = 1024         # q tokens
S = 1024         # kv tokens
D = 2048         # model dim
DK = D // P      # 16 contraction tiles
CH = 512         # t/s chunk (psum free size)
NCH = T // CH    # 2
HQ = 4           # q heads per core
ST = S // P      # 8 s-tiles
N_CORES = 8

F32 = mybir.dt.float32
F32R = mybir.dt.float32r
BF16 = mybir.dt.bfloat16
F16 = mybir.dt.float16

_CACHE = {}
LAST_RUN = {}


def _r(ap):
    return ap.bitcast(F32R)


def _build_program():
    nc = bacc.Bacc("TRN2", target_bir_lowering=False, debug=False, num_devices=1)

    xqT = nc.dram_tensor("xqT", [D, T], F32R, kind="ExternalInput").ap()
    xkvT = nc.dram_tensor("xkvT", [D, S], F32R, kind="ExternalInput").ap()
    wq = nc.dram_tensor("wq", [D, HQ * P], F32R, kind="ExternalInput").ap()
    wk = nc.dram_tensor("wk", [D, P], F32R, kind="ExternalInput").ap()
    wv = nc.dram_tensor("wv", [D, P], F32R, kind="ExternalInput").ap()
    wo = nc.dram_tensor("wo", [HQ * P, D], F16, kind="ExternalInput").ap()
    cosq = nc.dram_tensor("cosq", [P, T], F32, kind="ExternalInput").ap()
    sinq = nc.dram_tensor("sinq", [P, T], F32, kind="ExternalInput").ap()
    cosk = nc.dram_tensor("cosk", [P, S], F32, kind="ExternalInput").ap()
    sink = nc.dram_tensor("sink", [P, S], F32, kind="ExternalInput").ap()
    out = nc.dram_tensor("out", [T, D], F16, kind="ExternalOutput").ap()

    xqT_t = xqT.rearrange("(dk p) t -> p dk t", p=P)
    xkvT_t = xkvT.rearrange("(dk p) t -> p dk t", p=P)
    wq_t = wq.rearrange("(dk p) h -> p dk h", p=P)
    wk_t = wk.rearrange("(dk p) h -> p dk h", p=P)
    wv_t = wv.rearrange("(dk p) h -> p dk h", p=P)
    wo_t = wo.rearrange("(h p) o -> p h o", p=P)

    with tile.TileContext(nc) as tc, ExitStack() as ctx:
        xp = ctx.enter_context(tc.tile_pool(name="xp", bufs=4))
        wp = ctx.enter_context(tc.tile_pool(name="wp", bufs=1))
        kvp = ctx.enter_context(tc.tile_pool(name="kvp", bufs=1))
        qp = ctx.enter_context(tc.tile_pool(name="qp", bufs=2))
        ep = ctx.enter_context(tc.tile_pool(name="ep", bufs=2))
        op = ctx.enter_context(tc.tile_pool(name="op", bufs=2))
        outp = ctx.enter_context(tc.tile_pool(name="outp", bufs=3))
        sp = ctx.enter_context(tc.tile_pool(name="sp", bufs=2))
        ps_mm = ctx.enter_context(tc.tile_pool(name="ps_mm", bufs=3, space="PSUM"))
        ps_sc = ctx.enter_context(tc.tile_pool(name="ps_sc", bufs=3, space="PSUM"))
        ps_sum = ctx.enter_context(tc.tile_pool(name="ps_sum", bufs=2, space="PSUM"))

        # ---- persistent small tensors ----
        wk_sb = wp.tile([P, DK, P], F32R)
        nc.sync.dma_start(wk_sb[:], wk_t)
        wv_sb = wp.tile([P, DK, P], F32R)
        nc.sync.dma_start(wv_sb[:], wv_t)
        cosk_sb = wp.tile([P, S], F32)
        nc.sync.dma_start(cosk_sb[:], cosk)
        sink_sb = wp.tile([P, S], F32)
        nc.sync.dma_start(sink_sb[:], sink)
        ident = wp.tile([P, P], BF16)
        make_identity(nc, ident[:])
        ones_sb = wp.tile([P, 1], BF16)
        nc.gpsimd.memset(ones_sb[:], 1.0)

        ktrot = kvp.tile([P, S], F32R)       # K^T after rope: [h', s]
        v_sb = kvp.tile([P, ST, P], BF16)   # V tiles: [s_in_tile, s_tile, h']

        def rope(psrc, cos_sb, sin_sb, dst, c):
            """psrc: PSUM [P, CH] pre-rope [h',t]; writes rot into dst [P,CH]."""
            q_sb = qp.tile([P, CH], F32, tag="rope_in")
            nc.scalar.copy(q_sb[:], psrc[:])
            qsw = qp.tile([P, CH], F32, tag="rope_sw")
            nc.sync.dma_start(qsw[0:64, :], q_sb[64:128, :])
            nc.sync.dma_start(qsw[64:128, :], q_sb[0:64, :])
            t1 = qp.tile([P, CH], F32, tag="rope_t1")
            nc.vector.tensor_mul(t1[:], q_sb[:], cos_sb[:, bass.ts(c, CH)])
            nc.vector.tensor_mul(qsw[:], qsw[:], sin_sb[:, bass.ts(c, CH)])
            nc.vector.tensor_add(dst, t1[:], qsw[:])

        # ---- phase 0: K and V ----
        DQ = 4  # dk-tiles per streamed x slab
        for c in range(NCH):
            xk = [xp.tile([P, DQ, CH], F32R, tag="x", name=f"xk{c}_{i}") for i in range(DK // DQ)]
            for i, xt in enumerate(xk):
                nc.sync.dma_start(xt[:], xkvT_t[:, bass.ts(i, DQ), bass.ts(c, CH)])

            kps = ps_mm.tile([P, CH], F32, tag="mm")
            for dk in range(DK):
                nc.tensor.matmul(
                    kps[:], wk_sb[:, dk, :], xk[dk // DQ][:, dk % DQ, :],
                    start=(dk == 0), stop=(dk == DK - 1),
                )
            rope(kps, cosk_sb, sink_sb, ktrot[:, bass.ts(c, CH)], c)

            vps = ps_mm.tile([P, CH], F32, tag="mm")
            for dk in range(DK):
                nc.tensor.matmul(
                    vps[:], wv_sb[:, dk, :], xk[dk // DQ][:, dk % DQ, :],
                    start=(dk == 0), stop=(dk == DK - 1),
                )
            vt_sb = qp.tile([P, CH], BF16, tag="vt")
            nc.any.tensor_copy(vt_sb[:], vps[:])
            for i in range(CH // P):
                tps = ps_mm.tile([P, P], BF16, tag="mm")
                nc.tensor.transpose(tps[:], vt_sb[:, bass.ts(i, P)], ident[:])
                nc.vector.tensor_copy(v_sb[:, c * (CH // P) + i, :], tps[:])

        # ---- phase 1 persistent loads ----
        wq_sb = wp.tile([P, DK, HQ * P], F32R)
        nc.sync.dma_start(wq_sb[:], wq_t)
        wo_sb = wp.tile([P, HQ, D], F16)
        nc.sync.dma_start(wo_sb[:], wo_t)
        cosq_sb = wp.tile([P, T], F32)
        nc.sync.dma_start(cosq_sb[:], cosq)
        sinq_sb = wp.tile([P, T], F32)
        nc.sync.dma_start(sinq_sb[:], sinq)

        # ---- phase 1: Q, attention, output projection ----
        for c in range(NCH):
            xq = [xp.tile([P, DQ, CH], F32R, tag="x", name=f"xq{c}_{i}") for i in range(DK // DQ)]
            for i, xt in enumerate(xq):
                nc.sync.dma_start(xt[:], xqT_t[:, bass.ts(i, DQ), bass.ts(c, CH)])

            ot = op.tile([P, HQ, CH], F16, tag="ot")  # O^T for this chunk
            for h in range(HQ):
                qps = ps_mm.tile([P, CH], F32, tag="mm")
                for dk in range(DK):
                    nc.tensor.matmul(
                        qps[:], wq_sb[:, dk, bass.ts(h, P)],
                        xq[dk // DQ][:, dk % DQ, :],
                        start=(dk == 0), stop=(dk == DK - 1),
                    )
                qtrot = qp.tile([P, CH], F32R, tag="qtrot")
                rope(qps, cosq_sb, sinq_sb, qtrot[:], c)

                exps = ep.tile([P, ST, CH], BF16, tag="exps")
                sums_ps = ps_sum.tile([1, CH], F32, tag="sum")
                for st in range(ST):
                    sps = ps_sc.tile([P, CH], F32, tag="sc")
                    nc.tensor.matmul(
                        sps[:], ktrot[:, bass.ts(st, P)], qtrot[:],
                        start=True, stop=True,
                    )
                    nc.scalar.activation(
                        exps[:, st, :], sps[:], mybir.ActivationFunctionType.Exp
                    )
                    nc.tensor.matmul(
                        sums_ps[:], ones_sb[:], exps[:, st, :],
                        start=(st == 0), stop=(st == ST - 1),
                    )
                sums_sb = sp.tile([1, CH], F32, tag="sums")
                nc.vector.tensor_copy(sums_sb[:], sums_ps[:])
                recip = sp.tile([1, CH], F32, tag="recip")
                nc.vector.reciprocal(recip[:], sums_sb[:])
                rec_rep = qp.tile([P, CH], F32, tag="rec_rep")
                nc.gpsimd.partition_broadcast(rec_rep[:], recip[0:1, :])

                pv_ps = ps_mm.tile([P, CH], F32, tag="mm")
                for st in range(ST):
                    nc.tensor.matmul(
                        pv_ps[:], v_sb[:, st, :], exps[:, st, :],
                        start=(st == 0), stop=(st == ST - 1),
                    )
                nc.vector.tensor_mul(ot[:, h, :], pv_ps[:], rec_rep[:])

            for tt in range(CH // P):
                for oc in range(D // CH):
                    ops_ = ps_mm.tile([P, CH], F32, tag="mm")
                    for h in range(HQ):
                        nc.tensor.matmul(
                            ops_[:],
                            ot[:, h, bass.ts(tt, P)],
                            wo_sb[:, h, bass.ts(oc, CH)],
                            start=(h == 0), stop=(h == HQ - 1),
                        )
                    o_out = outp.tile([P, CH], F16, tag="oout")
                    nc.any.tensor_copy(o_out[:], ops_[:])
                    nc.sync.dma_start(
                        out[c * CH + tt * P : c * CH + (tt + 1) * P, bass.ts(oc, CH)],
                        o_out[:],
                    )

    nc.compile()
    return nc


def _rope_tables(positions):
    """positions: (L,) int32 -> cos [128, L], sin_signed [128, L] fp32."""
    half = P // 2
    j = np.arange(half, dtype=np.float64)
    timescale = 10000.0 ** (2.0 * j / P)
    ang = positions.astype(np.float64)[None, :] / timescale[:, None]  # (64, L)
    cos = np.cos(ang)
    sin = np.sin(ang)
    cos_t = np.concatenate([cos, cos], axis=0).astype(np.float32)
    sin_t = np.concatenate([-sin, sin], axis=0).astype(np.float32)
    return np.ascontiguousarray(cos_t), np.ascontiguousarray(sin_t)


def kernel(Xq, Xkv, q_positions, kv_positions, Wq, Wk, Wv, Wo, _trace=False):
    Xq = np.asarray(Xq, dtype=np.float32)
    Xkv = np.asarray(Xkv, dtype=np.float32)
    q_positions = np.asarray(q_positions)
    kv_positions = np.asarray(kv_positions)
    Wq = np.asarray(Wq, dtype=np.float32)
    Wk = np.asarray(Wk, dtype=np.float32)
    Wv = np.asarray(Wv, dtype=np.float32)
    Wo = np.asarray(Wo, dtype=np.float32)

    B = Xq.shape[0]
    G = N_CORES // B  # kv-head groups per batch

    if "nc" not in _CACHE:
        _CACHE["nc"] = _build_program()
    nc = _CACHE["nc"]

    in_maps = []
    for core in range(N_CORES):
        b, g = divmod(core, G)
        cos_q, sin_q = _rope_tables(q_positions[b])
        cos_k, sin_k = _rope_tables(kv_positions[b])
        in_maps.append({
            "xqT": np.ascontiguousarray(Xq[b].T),
            "xkvT": np.ascontiguousarray(Xkv[b].T),
            "wq": np.ascontiguousarray(Wq[:, g * HQ : (g + 1) * HQ, :].reshape(D, HQ * P)),
            "wk": np.ascontiguousarray(Wk[:, g, :]),
            "wv": np.ascontiguousarray(Wv[:, g, :]),
            "wo": np.ascontiguousarray(
                Wo[g * HQ : (g + 1) * HQ].reshape(HQ * P, D).astype(np.float16)
            ),
            "cosq": cos_q, "sinq": sin_q, "cosk": cos_k, "sink": sin_k,
        })

    r = run_bass_kernel_spmd(nc, in_maps, list(range(N_CORES)), trace=_trace)
    LAST_RUN["exec_time_ns"] = r.exec_time_ns
    LAST_RUN["mean_exec_time_ns"] = r.mean_exec_time_ns

    out = np.zeros((B, T, D), dtype=np.float32)
    for core in range(N_CORES):
        b = core // G
        out[b] += r.results[core]["out"].astype(np.float32)
    return out


# revision 18
# speedup vs baseline: 1.4895x; 1.4895x over previous
"""TRN2 Bass kernel for GQA attention (nn_Attention_13030930776201).

Reference computation (B=2, T=S=1024, D=2048, 16 Q heads / 4 KV heads, H=128):
    q = Xq @ Wq; k = Xkv @ Wk; v = Xkv @ Wv         (DenseGeneral projections)
    q, k = RoPE(q, q_pos), RoPE(k, kv_pos)
    out = softmax(q k^T) v  @ Wo                     (GQA, scale=1.0, no mask)

Sharding: 8 cores = 2 (batch) x 4 (KV-head group). Each core computes one
batch's attention for one KV head + its 4 Q heads, producing a partial
(1024, 2048) output; the host sums the 4 partials per batch.

Per-core dataflow (layouts chosen so NO on-device transposes of activations
are needed; the host passes X pre-transposed):
    QT[h',t]  = Wq[d,h'].T @ XqT[d,t]      (f32r matmuls, fp32 PSUM)
    KT[h',s]  = Wk[d,h'].T @ XkvT[d,s]
    VT[h',s]  = Wv[d,h'].T @ XkvT[d,s], PE-transposed to V[s,h'] (bf16)
    RoPE on QT/KT: half-swap via a PE permutation matmul + DVE muls (fp32)
    S^T[s,t]  = KT[h',s].T @ QT[h',t]      (f32r) ; expS = exp(S^T) (ACT, bf16)
    sums[t]   = ones[s].T @ expS[s,t]      (PE column-sum trick)
    O^T[h',t] = V[s,h'].T @ expS[s,t]      (bf16), normalized by 1/sums -> fp16
    out[t,o]  = O^T[h',t].T @ Wo[h',o]     (fp16), written as fp16 partial

float32r gives tf32-like precision at full PE rate (measured matmul rel err
1.5e-4 vs bf16's 2.4e-3), keeping softmax logits accurate; the V/O path is
precision-tolerant so it runs in bf16/fp16.

The emission order feeds the (serial, ~344 GB/s) DMA pipe just-in-time:
weights first, then X slabs in compute order; attention over chunk 0 is
split into s-halves so scores on the first KV chunk overlap the second
chunk's DMA + projections.
"""

import sys

if "/opt/trn_rl_repo" not in sys.path:
    sys.path.insert(0, "/opt/trn_rl_repo")

from contextlib import ExitStack

import numpy as np

import concourse.bass as bass
import concourse.tile as tile
from concourse import bacc, mybir
from concourse.bass_utils import run_bass_kernel_spmd
from concourse.masks import make_identity

P = 128          # partitions / head dim
T = 1024         # q tokens
S = 1024         # kv tokens
D = 2048         # model dim
DK = D // P      # 16 contraction tiles
CH = 512         # t/s chunk (psum free size)
NCH = T // CH    # 2
HQ = 4           # q heads per core
ST = S // P      # 8 s-tiles
HST = ST // 2    # s-tiles per half
N_CORES = 8

F32 = mybir.dt.float32
F32R = mybir.dt.float32r
BF16 = mybir.dt.bfloat16
F16 = mybir.dt.float16

_CACHE = {}
LAST_RUN = {}


def _build_program():
    nc = bacc.Bacc("TRN2", target_bir_lowering=False, debug=False, num_devices=1)

    xqT = nc.dram_tensor("xqT", [D, T], F32R, kind="ExternalInput").ap()
    xkvT = nc.dram_tensor("xkvT", [D, S], F32R, kind="ExternalInput").ap()
    wq = nc.dram_tensor("wq", [D, HQ * P], F32R, kind="ExternalInput").ap()
    wk = nc.dram_tensor("wk", [D, P], F32R, kind="ExternalInput").ap()
    wv = nc.dram_tensor("wv", [D, P], F32R, kind="ExternalInput").ap()
    wo = nc.dram_tensor("wo", [HQ * P, D], F16, kind="ExternalInput").ap()
    perm = nc.dram_tensor("perm", [P, P], F32R, kind="ExternalInput").ap()
    cosq = nc.dram_tensor("cosq", [P, T], F16, kind="ExternalInput").ap()
    sinq = nc.dram_tensor("sinq", [P, T], F16, kind="ExternalInput").ap()
    cosk = nc.dram_tensor("cosk", [P, S], F16, kind="ExternalInput").ap()
    sink = nc.dram_tensor("sink", [P, S], F16, kind="ExternalInput").ap()
    out = nc.dram_tensor("out", [T, D], F16, kind="ExternalOutput").ap()

    xqT_t = xqT.rearrange("(dk p) t -> p dk t", p=P)
    xkvT_t = xkvT.rearrange("(dk p) t -> p dk t", p=P)
    wq_t = wq.rearrange("(dk p) h -> p dk h", p=P)
    wk_t = wk.rearrange("(dk p) h -> p dk h", p=P)
    wv_t = wv.rearrange("(dk p) h -> p dk h", p=P)
    wo_t = wo.rearrange("(h p) o -> p h o", p=P)

    with tile.TileContext(nc) as tc, ExitStack() as ctx:
        xp = ctx.enter_context(tc.tile_pool(name="xp", bufs=6))
        wp = ctx.enter_context(tc.tile_pool(name="wp", bufs=1))
        rp = ctx.enter_context(tc.tile_pool(name="rp", bufs=2))
        kvp = ctx.enter_context(tc.tile_pool(name="kvp", bufs=1))
        qp = ctx.enter_context(tc.tile_pool(name="qp", bufs=2))
        qp1 = ctx.enter_context(tc.tile_pool(name="qp1", bufs=1))
        qtp = ctx.enter_context(tc.tile_pool(name="qtp", bufs=6))
        ep = ctx.enter_context(tc.tile_pool(name="ep", bufs=3))
        op = ctx.enter_context(tc.tile_pool(name="op", bufs=2))
        outp = ctx.enter_context(tc.tile_pool(name="outp", bufs=4))
        sp = ctx.enter_context(tc.tile_pool(name="sp", bufs=2))
        ps_mm = ctx.enter_context(tc.tile_pool(name="ps_mm", bufs=2, space="PSUM"))
        ps_qp = ctx.enter_context(tc.tile_pool(name="ps_qp", bufs=2, space="PSUM"))
        ps_sc = ctx.enter_context(tc.tile_pool(name="ps_sc", bufs=3, space="PSUM"))
        ps_sum = ctx.enter_context(tc.tile_pool(name="ps_sum", bufs=1, space="PSUM"))

        DQ = 4  # dk-tiles per streamed x slab
        NSL = DK // DQ  # 4 slabs per chunk

        def alloc_slab(pfx, i):
            return xp.tile([P, DQ, CH], F32R, tag="x", name=f"{pfx}_{i}")

        def dma_slab(t, x_t, c, i):
            nc.sync.dma_start(t[:], x_t[:, bass.ts(i, DQ), bass.ts(c, CH)])

        # ---- DMA order: wk -> xkv0 -> wv -> tables -> wq ->
        #      (xq0/xkv1 interleaved) -> wo -> xq1; out DMAs at the end ----
        perm_sb = wp.tile([P, P], F32R)
        nc.sync.dma_start(perm_sb[:], perm)
        wk_sb = rp.tile([P, DK, P], F32R, tag="recycle", name="wk_sb")
        for i in range(4):
            nc.sync.dma_start(wk_sb[:, bass.ts(i, DK // 4), :],
                              wk_t[:, bass.ts(i, DK // 4), :])
        ident = wp.tile([P, P], BF16)
        make_identity(nc, ident[:])
        ones_sb = wp.tile([P, 1], BF16)
        nc.gpsimd.memset(ones_sb[:], 1.0)

        xkv0 = [alloc_slab("xk0", i) for i in range(NSL)]
        for i, t in enumerate(xkv0):
            dma_slab(t, xkvT_t, 0, i)
        wv_sb = rp.tile([P, DK, P], F32R, tag="recycle", name="wv_sb")
        nc.sync.dma_start(wv_sb[:], wv_t)
        wq_sb = wp.tile([P, DK, HQ * P], F32R)
        for i in range(4):
            nc.sync.dma_start(wq_sb[:, bass.ts(i, DK // 4), :],
                              wq_t[:, bass.ts(i, DK // 4), :])
        cosk_sb = wp.tile([P, S], F16)
        nc.sync.dma_start(cosk_sb[:], cosk)
        sink_sb = wp.tile([P, S], F16)
        nc.sync.dma_start(sink_sb[:], sink)
        cosq_sb = wp.tile([P, T], F16)
        nc.sync.dma_start(cosq_sb[:], cosq)
        sinq_sb = wp.tile([P, T], F16)
        nc.sync.dma_start(sinq_sb[:], sinq)
        # interleave the xq-chunk0 / xkv-chunk1 slab loads so the PE gets
        # both Q-projection and KV-projection work per DMA'd megabyte
        xq0, xkv1 = [], []
        for i in range(NSL):
            t = alloc_slab("xq0", i)
            dma_slab(t, xqT_t, 0, i)
            xq0.append(t)
            t = alloc_slab("xk1", i)
            dma_slab(t, xkvT_t, 1, i)
            xkv1.append(t)

        ktrot = kvp.tile([P, S], F32R)       # K^T after rope: [h', s]
        v_sb = kvp.tile([P, ST, P], BF16)   # V tiles: [s_in_tile, s_tile, h']

        def rope(psrc, cos_sb, sin_sb, dst, c, nm):
            """psrc: PSUM [P, CH] pre-rope [h',t]; writes rot into dst [P,CH].

            The half-swap q[(i+64)%128] comes from a PE permutation matmul so
            the DMA pipe stays free for HBM traffic.
            """
            q_sb = qp.tile([P, CH], F32R, tag="rope_in")
            nc.scalar.copy(q_sb[:], psrc[:])
            sw_ps = ps_sc.tile([P, CH], F32, tag="sc", name=f"swp_{nm}")
            nc.tensor.matmul(sw_ps[:], perm_sb[:], q_sb[:], start=True, stop=True)
            t1 = qp1.tile([P, CH], F32, tag="rope_t1")
            nc.vector.tensor_mul(t1[:], q_sb[:], cos_sb[:, bass.ts(c, CH)])
            t2 = qp1.tile([P, CH], F32, tag="rope_t2")
            nc.vector.tensor_mul(t2[:], sw_ps[:], sin_sb[:, bass.ts(c, CH)])
            nc.vector.tensor_add(dst, t1[:], t2[:])

        def proj(w_ap, slabs, pool, tag, name):
            ps = pool.tile([P, CH], F32, tag=tag, name=name)
            for dk in range(DK):
                nc.tensor.matmul(
                    ps[:], w_ap[:, dk, :], slabs[dk // DQ][:, dk % DQ, :],
                    start=(dk == 0), stop=(dk == DK - 1),
                )
            return ps

        def kv_chunk(c, xk):
            kps = proj(wk_sb, xk, ps_mm, "mm", f"kps{c}")
            rope(kps, cosk_sb, sink_sb, ktrot[:, bass.ts(c, CH)], c, f"k{c}")
            vps = proj(wv_sb, xk, ps_mm, "mm", f"vps{c}")
            vt_sb = qp1.tile([P, CH], BF16, tag="vt")
            nc.any.tensor_copy(vt_sb[:], vps[:])
            for i in range(CH // P):
                tps = ps_mm.tile([P, P], BF16, tag="mm", name=f"tps{c}_{i}")
                nc.tensor.transpose(tps[:], vt_sb[:, bass.ts(i, P)], ident[:])
                nc.vector.tensor_copy(v_sb[:, c * (CH // P) + i, :], tps[:])

        qtrot = {}
        exps_tiles = {}

        def q_chunk(c, xq):
            for h in range(HQ):
                qps = ps_qp.tile([P, CH], F32, tag="qp", name=f"qps{c}_{h}")
                for dk in range(DK):
                    nc.tensor.matmul(
                        qps[:], wq_sb[:, dk, bass.ts(h, P)],
                        xq[dk // DQ][:, dk % DQ, :],
                        start=(dk == 0), stop=(dk == DK - 1),
                    )
                qt = qtp.tile([P, CH], F32R, tag="qtrot", name=f"qtrot{c}_{h}")
                rope(qps, cosq_sb, sinq_sb, qt[:], c, f"q{c}_{h}")
                qtrot[(c, h)] = qt

        def scores_half(c, h, half):
            """Scores+exp+colsum for s-tiles in `half`; returns sums_sb."""
            if (c, h) not in exps_tiles:
                exps_tiles[(c, h)] = ep.tile(
                    [P, ST, CH], BF16, tag="exps", name=f"exps{c}_{h}")
            exps = exps_tiles[(c, h)]
            qt = qtrot[(c, h)]
            sums_ps = ps_sum.tile([1, CH], F32, tag="sum",
                                  name=f"sums{c}_{h}_{half}")
            for j in range(HST):
                st = half * HST + j
                sps = ps_sc.tile([P, CH], F32, tag="sc", name=f"sps{c}_{h}_{st}")
                nc.tensor.matmul(
                    sps[:], ktrot[:, bass.ts(st, P)], qt[:],
                    start=True, stop=True,
                )
                nc.scalar.activation(
                    exps[:, st, :], sps[:], mybir.ActivationFunctionType.Exp
                )
                nc.tensor.matmul(
                    sums_ps[:], ones_sb[:], exps[:, st, :],
                    start=(j == 0), stop=(j == HST - 1),
                )
            sums_sb = sp.tile([1, CH], F32, tag=f"sums{half}",
                              name=f"sumsb{c}_{h}_{half}")
            nc.vector.tensor_copy(sums_sb[:], sums_ps[:])
            return sums_sb

        def attn_finish(c, h, ot, sums_list):
            exps = exps_tiles.pop((c, h))
            if len(sums_list) > 1:
                tot = sp.tile([1, CH], F32, tag="sumtot", name=f"sumt{c}_{h}")
                nc.vector.tensor_add(tot[:], sums_list[0][:], sums_list[1][:])
            else:
                tot = sums_list[0]
            recip = sp.tile([1, CH], F32, tag="recip", name=f"recip{c}_{h}")
            nc.vector.reciprocal(recip[:], tot[:])
            rec_rep = qp.tile([P, CH], F32, tag="rec_rep")
            nc.gpsimd.partition_broadcast(rec_rep[:], recip[0:1, :])

            pv_ps = ps_mm.tile([P, CH], F32, tag="mm", name=f"pv{c}_{h}")
            for st in range(ST):
                nc.tensor.matmul(
                    pv_ps[:], v_sb[:, st, :], exps[:, st, :],
                    start=(st == 0), stop=(st == ST - 1),
                )
            nc.vector.tensor_mul(ot[:, h, :], pv_ps[:], rec_rep[:])

        def oproj(c, ot, wo_tiles):
            for tt in range(CH // P):
                for oc in range(D // CH):
                    ops_ = ps_mm.tile([P, CH], F32, tag="mm",
                                      name=f"op{c}_{tt}_{oc}")
                    for h in range(HQ):
                        nc.tensor.matmul(
                            ops_[:],
                            ot[:, h, bass.ts(tt, P)],
                            wo_tiles[h // 2][:, h % 2, bass.ts(oc, CH)],
                            start=(h == 0), stop=(h == HQ - 1),
                        )
                    o_out = outp.tile([P, CH], F16, tag="oout")
                    nc.any.tensor_copy(o_out[:], ops_[:])
                    nc.sync.dma_start(
                        out[c * CH + tt * P : c * CH + (tt + 1) * P,
                            bass.ts(oc, CH)],
                        o_out[:],
                    )

        # wo reuses the wk/wv SBUF slots (dead after the chunk-1
        # projections); loaded after xq0, before xq1
        wo_tiles = [rp.tile([P, 2, D], F16, tag="recycle", name=f"wo{i}")
                    for i in range(2)]
        for i in range(2):
            nc.sync.dma_start(wo_tiles[i][:], wo_t[:, bass.ts(i, 2), :])
        xq1 = [alloc_slab("xq1", i) for i in range(NSL)]
        for i, t in enumerate(xq1):
            dma_slab(t, xqT_t, 1, i)

        # ================= compute emission =================
        kv_chunk(0, xkv0)
        kv_chunk(1, xkv1)
        q_chunk(0, xq0)
        ot0 = op.tile([P, HQ, CH], F16, tag="ot", name="ot0")
        for h in range(HQ):
            sa = scores_half(0, h, 0)
            sb = scores_half(0, h, 1)
            attn_finish(0, h, ot0, [sa, sb])
        oproj(0, ot0, wo_tiles)
        q_chunk(1, xq1)
        ot1 = op.tile([P, HQ, CH], F16, tag="ot", name="ot1")
        for h in range(HQ):
            sa = scores_half(1, h, 0)
            sb = scores_half(1, h, 1)
            attn_finish(1, h, ot1, [sa, sb])
        oproj(1, ot1, wo_tiles)

    nc.compile()
    return nc


def _rope_tables(positions):
    """positions: (L,) int -> cos [128, L], sin_signed [128, L] fp16."""
    half = P // 2
    j = np.arange(half, dtype=np.float64)
    timescale = 10000.0 ** (2.0 * j / P)
    ang = positions.astype(np.float64)[None, :] / timescale[:, None]  # (64, L)
    cos = np.cos(ang)
    sin = np.sin(ang)
    cos_t = np.concatenate([cos, cos], axis=0).astype(np.float16)
    sin_t = np.concatenate([-sin, sin], axis=0).astype(np.float16)
    return np.ascontiguousarray(cos_t), np.ascontiguousarray(sin_t)


def kernel(Xq, Xkv, q_positions, kv_positions, Wq, Wk, Wv, Wo, _trace=False):
    Xq = np.asarray(Xq, dtype=np.float32)
    Xkv = np.asarray(Xkv, dtype=np.float32)
    q_positions = np.asarray(q_positions)
    kv_positions = np.asarray(kv_positions)
    Wq = np.asarray(Wq, dtype=np.float32)
    Wk = np.asarray(Wk, dtype=np.float32)
    Wv = np.asarray(Wv, dtype=np.float32)
    Wo = np.asarray(Wo, dtype=np.float32)

    B = Xq.shape[0]
    G = N_CORES // B  # kv-head groups per batch

    if "nc" not in _CACHE:
        _CACHE["nc"] = _build_program()
    nc = _CACHE["nc"]

    perm = np.ascontiguousarray(np.roll(np.eye(P, dtype=np.float32), P // 2, axis=0))

    in_maps = []
    for core in range(N_CORES):
        b, g = divmod(core, G)
        cos_q, sin_q = _rope_tables(q_positions[b])
        cos_k, sin_k = _rope_tables(kv_positions[b])
        in_maps.append({
            "xqT": np.ascontiguousarray(Xq[b].T),
            "xkvT": np.ascontiguousarray(Xkv[b].T),
            "wq": np.ascontiguousarray(
                Wq[:, g * HQ : (g + 1) * HQ, :].reshape(D, HQ * P)),
            "wk": np.ascontiguousarray(Wk[:, g, :]),
            "wv": np.ascontiguousarray(Wv[:, g, :]),
            "wo": np.ascontiguousarray(
                Wo[g * HQ : (g + 1) * HQ].reshape(HQ * P, D).astype(np.float16)),
            "perm": perm,
            "cosq": cos_q, "sinq": sin_q, "cosk": cos_k, "sink": sin_k,
        })

    r = run_bass_kernel_spmd(nc, in_maps, list(range(N_CORES)), trace=_trace)
    LAST_RUN["exec_time_ns"] = r.exec_time_ns
    LAST_RUN["mean_exec_time_ns"] = r.mean_exec_time_ns

    out = np.zeros((B, T, D), dtype=np.float32)
    for core in range(N_CORES):
        b = core // G
        out[b] += r.results[core]["out"].astype(np.float32)
    return out


# revision 21
# speedup vs baseline: 1.4970x; 1.0050x over previous
"""TRN2 Bass kernel for GQA attention (nn_Attention_13030930776201).

Reference computation (B=2, T=S=1024, D=2048, 16 Q heads / 4 KV heads, H=128):
    q = Xq @ Wq; k = Xkv @ Wk; v = Xkv @ Wv         (DenseGeneral projections)
    q, k = RoPE(q, q_pos), RoPE(k, kv_pos)
    out = softmax(q k^T) v  @ Wo                     (GQA, scale=1.0, no mask)

Sharding: 8 cores = 2 (batch) x 4 (KV-head group). Each core computes one
batch's attention for one KV head + its 4 Q heads, producing a partial
(1024, 2048) output; the host sums the 4 partials per batch.

Per-core dataflow (layouts chosen so NO on-device transposes of activations
are needed; the host passes X pre-transposed):
    QT[h',t]  = Wq[d,h'].T @ XqT[d,t]      (f32r matmuls, fp32 PSUM)
    KT[h',s]  = Wk[d,h'].T @ XkvT[d,s]
    VT[h',s]  = Wv[d,h'].T @ XkvT[d,s], PE-transposed to V[s,h'] (bf16)
    RoPE on QT/KT: half-swap via a PE permutation matmul + DVE muls (fp32)
    S^T[s,t]  = KT[h',s].T @ QT[h',t]      (f32r) ; expS = exp(S^T) (ACT, bf16)
    sums[t]   = ones[s].T @ expS[s,t]      (PE column-sum trick)
    O^T[h',t] = V[s,h'].T @ expS[s,t]      (bf16), normalized by 1/sums -> fp16
    out[t,o]  = O^T[h',t].T @ Wo[h',o]     (fp16), written as fp16 partial

float32r gives tf32-like precision at full PE rate (measured matmul rel err
1.5e-4 vs bf16's 2.4e-3), keeping softmax logits accurate; the V/O path is
precision-tolerant so it runs in bf16/fp16.

The emission order feeds the (serial, ~344 GB/s) DMA pipe just-in-time:
weights first, then X slabs in compute order; attention over chunk 0 is
split into s-halves so scores on the first KV chunk overlap the second
chunk's DMA + projections.
"""

import sys

if "/opt/trn_rl_repo" not in sys.path:
    sys.path.insert(0, "/opt/trn_rl_repo")

from contextlib import ExitStack

import numpy as np

import concourse.bass as bass
import concourse.tile as tile
from concourse import bacc, mybir
from concourse.bass_utils import run_bass_kernel_spmd
from concourse.masks import make_identity

P = 128          # partitions / head dim
T = 1024         # q tokens
S = 1024         # kv tokens
D = 2048         # model dim
DK = D // P      # 16 contraction tiles
CH = 512         # t/s chunk (psum free size)
NCH = T // CH    # 2
HQ = 4           # q heads per core
ST = S // P      # 8 s-tiles
HST = ST // 2    # s-tiles per half
N_CORES = 8

F32 = mybir.dt.float32
F32R = mybir.dt.float32r
BF16 = mybir.dt.bfloat16
F16 = mybir.dt.float16

_CACHE = {}
LAST_RUN = {}


def _build_program():
    nc = bacc.Bacc("TRN2", target_bir_lowering=False, debug=False, num_devices=1)

    xqT = nc.dram_tensor("xqT", [D, T], F32R, kind="ExternalInput").ap()
    xkvT = nc.dram_tensor("xkvT", [D, S], F32R, kind="ExternalInput").ap()
    wq = nc.dram_tensor("wq", [D, HQ * P], F32R, kind="ExternalInput").ap()
    wk = nc.dram_tensor("wk", [D, P], F32R, kind="ExternalInput").ap()
    wv = nc.dram_tensor("wv", [D, P], F32R, kind="ExternalInput").ap()
    wo = nc.dram_tensor("wo", [HQ * P, D], F16, kind="ExternalInput").ap()
    perm = nc.dram_tensor("perm", [P, P], F32R, kind="ExternalInput").ap()
    cosq = nc.dram_tensor("cosq", [P, T], F16, kind="ExternalInput").ap()
    sinq = nc.dram_tensor("sinq", [P, T], F16, kind="ExternalInput").ap()
    cosk = nc.dram_tensor("cosk", [P, S], F16, kind="ExternalInput").ap()
    sink = nc.dram_tensor("sink", [P, S], F16, kind="ExternalInput").ap()
    out = nc.dram_tensor("out", [T, D], F16, kind="ExternalOutput").ap()

    xqT_t = xqT.rearrange("(dk p) t -> p dk t", p=P)
    xkvT_t = xkvT.rearrange("(dk p) t -> p dk t", p=P)
    wq_t = wq.rearrange("(dk p) h -> p dk h", p=P)
    wk_t = wk.rearrange("(dk p) h -> p dk h", p=P)
    wv_t = wv.rearrange("(dk p) h -> p dk h", p=P)
    wo_t = wo.rearrange("(h p) o -> p h o", p=P)

    with tile.TileContext(nc) as tc, ExitStack() as ctx:
        xp = ctx.enter_context(tc.tile_pool(name="xp", bufs=6))
        wp = ctx.enter_context(tc.tile_pool(name="wp", bufs=1))
        rp = ctx.enter_context(tc.tile_pool(name="rp", bufs=2))
        kvp = ctx.enter_context(tc.tile_pool(name="kvp", bufs=1))
        qp = ctx.enter_context(tc.tile_pool(name="qp", bufs=2))
        qp1 = ctx.enter_context(tc.tile_pool(name="qp1", bufs=1))
        qtp = ctx.enter_context(tc.tile_pool(name="qtp", bufs=6))
        ep = ctx.enter_context(tc.tile_pool(name="ep", bufs=3))
        op = ctx.enter_context(tc.tile_pool(name="op", bufs=2))
        outp = ctx.enter_context(tc.tile_pool(name="outp", bufs=4))
        sp = ctx.enter_context(tc.tile_pool(name="sp", bufs=2))
        ps_mm = ctx.enter_context(tc.tile_pool(name="ps_mm", bufs=2, space="PSUM"))
        ps_qp = ctx.enter_context(tc.tile_pool(name="ps_qp", bufs=2, space="PSUM"))
        ps_sc = ctx.enter_context(tc.tile_pool(name="ps_sc", bufs=3, space="PSUM"))
        ps_sum = ctx.enter_context(tc.tile_pool(name="ps_sum", bufs=1, space="PSUM"))

        DQ = 4  # dk-tiles per streamed x slab
        NSL = DK // DQ  # 4 slabs per chunk

        def alloc_slab(pfx, i):
            return xp.tile([P, DQ, CH], F32R, tag="x", name=f"{pfx}_{i}")

        def dma_slab(t, x_t, c, i):
            nc.sync.dma_start(t[:], x_t[:, bass.ts(i, DQ), bass.ts(c, CH)])

        # ---- DMA order: wk -> xkv0 -> wv -> tables -> wq ->
        #      (xq0/xkv1 interleaved) -> wo -> xq1; out DMAs at the end ----
        perm_sb = wp.tile([P, P], F32R)
        nc.sync.dma_start(perm_sb[:], perm)
        wk_sb = rp.tile([P, DK, P], F32R, tag="recycle", name="wk_sb")
        for i in range(4):
            nc.sync.dma_start(wk_sb[:, bass.ts(i, DK // 4), :],
                              wk_t[:, bass.ts(i, DK // 4), :])
        ident = wp.tile([P, P], BF16)
        make_identity(nc, ident[:])
        ones_sb = wp.tile([P, 1], BF16)
        nc.gpsimd.memset(ones_sb[:], 1.0)

        xkv0 = [alloc_slab("xk0", i) for i in range(NSL)]
        for i, t in enumerate(xkv0):
            dma_slab(t, xkvT_t, 0, i)
        wq_sb = wp.tile([P, DK, HQ * P], F32R)
        for i in range(4):
            nc.sync.dma_start(wq_sb[:, bass.ts(i, DK // 4), :],
                              wq_t[:, bass.ts(i, DK // 4), :])
        wv_sb = rp.tile([P, DK, P], F32R, tag="recycle", name="wv_sb")
        nc.sync.dma_start(wv_sb[:], wv_t)
        cosk_sb = wp.tile([P, S], F16)
        nc.sync.dma_start(cosk_sb[:], cosk)
        sink_sb = wp.tile([P, S], F16)
        nc.sync.dma_start(sink_sb[:], sink)
        cosq_sb = wp.tile([P, T], F16)
        nc.sync.dma_start(cosq_sb[:], cosq)
        sinq_sb = wp.tile([P, T], F16)
        nc.sync.dma_start(sinq_sb[:], sinq)
        # interleave the xq-chunk0 / xkv-chunk1 slab loads so the PE gets
        # both Q-projection and KV-projection work per DMA'd megabyte
        xq0, xkv1 = [], []
        for i in range(NSL):
            t = alloc_slab("xq0", i)
            dma_slab(t, xqT_t, 0, i)
            xq0.append(t)
            t = alloc_slab("xk1", i)
            dma_slab(t, xkvT_t, 1, i)
            xkv1.append(t)

        ktrot = kvp.tile([P, S], F32R)       # K^T after rope: [h', s]
        v_sb = kvp.tile([P, ST, P], BF16)   # V tiles: [s_in_tile, s_tile, h']

        def rope(psrc, cos_sb, sin_sb, dst, c, nm):
            """psrc: PSUM [P, CH] pre-rope [h',t]; writes rot into dst [P,CH].

            The half-swap q[(i+64)%128] comes from a PE permutation matmul so
            the DMA pipe stays free for HBM traffic.
            """
            q_sb = qp.tile([P, CH], F32R, tag="rope_in")
            nc.scalar.copy(q_sb[:], psrc[:])
            sw_ps = ps_sc.tile([P, CH], F32, tag="sc", name=f"swp_{nm}")
            nc.tensor.matmul(sw_ps[:], perm_sb[:], q_sb[:], start=True, stop=True)
            t1 = qp1.tile([P, CH], F32, tag="rope_t1")
            nc.vector.tensor_mul(t1[:], q_sb[:], cos_sb[:, bass.ts(c, CH)])
            t2 = qp1.tile([P, CH], F32, tag="rope_t2")
            nc.vector.tensor_mul(t2[:], sw_ps[:], sin_sb[:, bass.ts(c, CH)])
            nc.vector.tensor_add(dst, t1[:], t2[:])

        def proj(w_ap, slabs, pool, tag, name):
            ps = pool.tile([P, CH], F32, tag=tag, name=name)
            for dk in range(DK):
                nc.tensor.matmul(
                    ps[:], w_ap[:, dk, :], slabs[dk // DQ][:, dk % DQ, :],
                    start=(dk == 0), stop=(dk == DK - 1),
                )
            return ps

        def kv_chunk(c, xk):
            kps = proj(wk_sb, xk, ps_mm, "mm", f"kps{c}")
            rope(kps, cosk_sb, sink_sb, ktrot[:, bass.ts(c, CH)], c, f"k{c}")
            vps = proj(wv_sb, xk, ps_mm, "mm", f"vps{c}")
            vt_sb = qp1.tile([P, CH], BF16, tag="vt")
            nc.any.tensor_copy(vt_sb[:], vps[:])
            for i in range(CH // P):
                tps = ps_mm.tile([P, P], BF16, tag="mm", name=f"tps{c}_{i}")
                nc.tensor.transpose(tps[:], vt_sb[:, bass.ts(i, P)], ident[:])
                nc.vector.tensor_copy(v_sb[:, c * (CH // P) + i, :], tps[:])

        qtrot = {}
        exps_tiles = {}

        def q_chunk(c, xq):
            for h in range(HQ):
                qps = ps_qp.tile([P, CH], F32, tag="qp", name=f"qps{c}_{h}")
                for dk in range(DK):
                    nc.tensor.matmul(
                        qps[:], wq_sb[:, dk, bass.ts(h, P)],
                        xq[dk // DQ][:, dk % DQ, :],
                        start=(dk == 0), stop=(dk == DK - 1),
                    )
                qt = qtp.tile([P, CH], F32R, tag="qtrot", name=f"qtrot{c}_{h}")
                rope(qps, cosq_sb, sinq_sb, qt[:], c, f"q{c}_{h}")
                qtrot[(c, h)] = qt

        def scores_half(c, h, half):
            """Scores+exp+colsum for s-tiles in `half`; returns sums_sb."""
            if (c, h) not in exps_tiles:
                exps_tiles[(c, h)] = ep.tile(
                    [P, ST, CH], BF16, tag="exps", name=f"exps{c}_{h}")
            exps = exps_tiles[(c, h)]
            qt = qtrot[(c, h)]
            sums_ps = ps_sum.tile([1, CH], F32, tag="sum",
                                  name=f"sums{c}_{h}_{half}")
            # scores first (consecutive MMs), then the colsum accumulation
            # reusing the constant ones lhsT back-to-back
            sps_tiles = []
            for j in range(HST):
                st = half * HST + j
                sps = ps_sc.tile([P, CH], F32, tag="sc", name=f"sps{c}_{h}_{st}")
                nc.tensor.matmul(
                    sps[:], ktrot[:, bass.ts(st, P)], qt[:],
                    start=True, stop=True,
                )
                nc.scalar.activation(
                    exps[:, st, :], sps[:], mybir.ActivationFunctionType.Exp
                )
            for j in range(HST):
                st = half * HST + j
                nc.tensor.matmul(
                    sums_ps[:], ones_sb[:], exps[:, st, :],
                    start=(j == 0), stop=(j == HST - 1),
                )
            sums_sb = sp.tile([1, CH], F32, tag=f"sums{half}",
                              name=f"sumsb{c}_{h}_{half}")
            nc.vector.tensor_copy(sums_sb[:], sums_ps[:])
            return sums_sb

        def attn_finish(c, h, ot, sums_list):
            exps = exps_tiles.pop((c, h))
            if len(sums_list) > 1:
                tot = sp.tile([1, CH], F32, tag="sumtot", name=f"sumt{c}_{h}")
                nc.vector.tensor_add(tot[:], sums_list[0][:], sums_list[1][:])
            else:
                tot = sums_list[0]
            recip = sp.tile([1, CH], F32, tag="recip", name=f"recip{c}_{h}")
            nc.vector.reciprocal(recip[:], tot[:])
            rec_rep = qp.tile([P, CH], F32, tag="rec_rep")
            nc.gpsimd.partition_broadcast(rec_rep[:], recip[0:1, :])

            pv_ps = ps_mm.tile([P, CH], F32, tag="mm", name=f"pv{c}_{h}")
            for st in range(ST):
                nc.tensor.matmul(
                    pv_ps[:], v_sb[:, st, :], exps[:, st, :],
                    start=(st == 0), stop=(st == ST - 1),
                )
            nc.vector.tensor_mul(ot[:, h, :], pv_ps[:], rec_rep[:])

        def oproj(c, ot, wo_tiles):
            for tt in range(CH // P):
                for oc in range(D // CH):
                    ops_ = ps_mm.tile([P, CH], F32, tag="mm",
                                      name=f"op{c}_{tt}_{oc}")
                    for h in range(HQ):
                        nc.tensor.matmul(
                            ops_[:],
                            ot[:, h, bass.ts(tt, P)],
                            wo_tiles[h // 2][:, h % 2, bass.ts(oc, CH)],
                            start=(h == 0), stop=(h == HQ - 1),
                        )
                    o_out = outp.tile([P, CH], F16, tag="oout")
                    nc.any.tensor_copy(o_out[:], ops_[:])
                    nc.sync.dma_start(
                        out[c * CH + tt * P : c * CH + (tt + 1) * P,
                            bass.ts(oc, CH)],
                        o_out[:],
                    )

        # wo reuses the wk/wv SBUF slots (dead after the chunk-1
        # projections); loaded after xq0, before xq1
        wo_tiles = [rp.tile([P, 2, D], F16, tag="recycle", name=f"wo{i}")
                    for i in range(2)]
        for i in range(2):
            nc.sync.dma_start(wo_tiles[i][:], wo_t[:, bass.ts(i, 2), :])
        xq1 = [alloc_slab("xq1", i) for i in range(NSL)]
        for i, t in enumerate(xq1):
            dma_slab(t, xqT_t, 1, i)

        # ================= compute emission =================
        kv_chunk(0, xkv0)
        kv_chunk(1, xkv1)
        q_chunk(0, xq0)
        ot0 = op.tile([P, HQ, CH], F16, tag="ot", name="ot0")
        for h in range(HQ):
            sa = scores_half(0, h, 0)
            sb = scores_half(0, h, 1)
            attn_finish(0, h, ot0, [sa, sb])
        oproj(0, ot0, wo_tiles)
        q_chunk(1, xq1)
        ot1 = op.tile([P, HQ, CH], F16, tag="ot", name="ot1")
        for h in range(HQ):
            sa = scores_half(1, h, 0)
            sb = scores_half(1, h, 1)
            attn_finish(1, h, ot1, [sa, sb])
        oproj(1, ot1, wo_tiles)

    nc.compile()
    return nc


def _rope_tables(positions):
    """positions: (L,) int -> cos [128, L], sin_signed [128, L] fp16."""
    half = P // 2
    j = np.arange(half, dtype=np.float64)
    timescale = 10000.0 ** (2.0 * j / P)
    ang = positions.astype(np.float64)[None, :] / timescale[:, None]  # (64, L)
    cos = np.cos(ang)
    sin = np.sin(ang)
    cos_t = np.concatenate([cos, cos], axis=0).astype(np.float16)
    sin_t = np.concatenate([-sin, sin], axis=0).astype(np.float16)
    return np.ascontiguousarray(cos_t), np.ascontiguousarray(sin_t)


def kernel(Xq, Xkv, q_positions, kv_positions, Wq, Wk, Wv, Wo, _trace=False):
    Xq = np.asarray(Xq, dtype=np.float32)
    Xkv = np.asarray(Xkv, dtype=np.float32)
    q_positions = np.asarray(q_positions)
    kv_positions = np.asarray(kv_positions)
    Wq = np.asarray(Wq, dtype=np.float32)
    Wk = np.asarray(Wk, dtype=np.float32)
    Wv = np.asarray(Wv, dtype=np.float32)
    Wo = np.asarray(Wo, dtype=np.float32)

    B = Xq.shape[0]
    G = N_CORES // B  # kv-head groups per batch

    if "nc" not in _CACHE:
        _CACHE["nc"] = _build_program()
    nc = _CACHE["nc"]

    perm = np.ascontiguousarray(np.roll(np.eye(P, dtype=np.float32), P // 2, axis=0))

    per_b = {}
    for b in range(B):
        cos_q, sin_q = _rope_tables(q_positions[b])
        cos_k, sin_k = _rope_tables(kv_positions[b])
        per_b[b] = (np.ascontiguousarray(Xq[b].T), np.ascontiguousarray(Xkv[b].T),
                    cos_q, sin_q, cos_k, sin_k)
    in_maps = []
    for core in range(N_CORES):
        b, g = divmod(core, G)
        xqT_b, xkvT_b, cos_q, sin_q, cos_k, sin_k = per_b[b]
        in_maps.append({
            "xqT": xqT_b,
            "xkvT": xkvT_b,
            "wq": np.ascontiguousarray(
                Wq[:, g * HQ : (g + 1) * HQ, :].reshape(D, HQ * P)),
            "wk": np.ascontiguousarray(Wk[:, g, :]),
            "wv": np.ascontiguousarray(Wv[:, g, :]),
            "wo": np.ascontiguousarray(
                Wo[g * HQ : (g + 1) * HQ].reshape(HQ * P, D).astype(np.float16)),
            "perm": perm,
            "cosq": cos_q, "sinq": sin_q, "cosk": cos_k, "sink": sin_k,
        })

    r = run_bass_kernel_spmd(nc, in_maps, list(range(N_CORES)), trace=_trace)
    LAST_RUN["exec_time_ns"] = r.exec_time_ns
    LAST_RUN["mean_exec_time_ns"] = r.mean_exec_time_ns

    out = np.zeros((B, T, D), dtype=np.float32)
    for core in range(N_CORES):
        b = core // G
        out[b] += r.results[core]["out"].astype(np.float32)
    return out


# revision 24
# speedup vs baseline: 1.5022x; 1.0034x over previous
"""TRN2 Bass kernel for GQA attention (nn_Attention_13030930776201).

Reference computation (B=2, T=S=1024, D=2048, 16 Q heads / 4 KV heads, H=128):
    q = Xq @ Wq; k = Xkv @ Wk; v = Xkv @ Wv         (DenseGeneral projections)
    q, k = RoPE(q, q_pos), RoPE(k, kv_pos)
    out = softmax(q k^T) v  @ Wo                     (GQA, scale=1.0, no mask)

Sharding: 8 cores = 2 (batch) x 4 (KV-head group). Each core computes one
batch's attention for one KV head + its 4 Q heads, producing a partial
(1024, 2048) output; the host sums the 4 partials per batch.

Per-core dataflow (layouts chosen so NO on-device transposes of activations
are needed; the host passes X pre-transposed):
    QT[h',t]  = Wq[d,h'].T @ XqT[d,t]      (f32r matmuls, fp32 PSUM)
    KT[h',s]  = Wk[d,h'].T @ XkvT[d,s]
    VT[h',s]  = Wv[d,h'].T @ XkvT[d,s], PE-transposed to V[s,h'] (bf16)
    RoPE on QT/KT: half-swap via a PE permutation matmul + DVE muls (fp32)
    S^T[s,t]  = KT[h',s].T @ QT[h',t]      (f32r) ; expS = exp(S^T) (ACT, bf16)
    sums[t]   = ones[s].T @ expS[s,t]      (PE column-sum trick)
    O^T[h',t] = V[s,h'].T @ expS[s,t]      (bf16), normalized by 1/sums -> fp16
    out[t,o]  = O^T[h',t].T @ Wo[h',o]     (fp16), written as fp16 partial

float32r gives tf32-like precision at full PE rate (measured matmul rel err
1.5e-4 vs bf16's 2.4e-3), keeping softmax logits accurate; the V/O path is
precision-tolerant so it runs in bf16/fp16.

The emission order feeds the (serial, ~344 GB/s) DMA pipe just-in-time:
weights first, then X slabs in compute order; attention over chunk 0 is
split into s-halves so scores on the first KV chunk overlap the second
chunk's DMA + projections.
"""

import sys

if "/opt/trn_rl_repo" not in sys.path:
    sys.path.insert(0, "/opt/trn_rl_repo")

from contextlib import ExitStack

import numpy as np

import concourse.bass as bass
import concourse.tile as tile
from concourse import bacc, mybir
from concourse.bass_utils import run_bass_kernel_spmd
from concourse.masks import make_identity

P = 128          # partitions / head dim
T = 1024         # q tokens
S = 1024         # kv tokens
D = 2048         # model dim
DK = D // P      # 16 contraction tiles
CH = 512         # t/s chunk (psum free size)
NCH = T // CH    # 2
HQ = 4           # q heads per core
ST = S // P      # 8 s-tiles
HST = ST // 2    # s-tiles per half
N_CORES = 8

F32 = mybir.dt.float32
F32R = mybir.dt.float32r
BF16 = mybir.dt.bfloat16
F16 = mybir.dt.float16

_CACHE = {}
LAST_RUN = {}


def _build_program():
    nc = bacc.Bacc("TRN2", target_bir_lowering=False, debug=False, num_devices=1)

    xqT = nc.dram_tensor("xqT", [D, T], F32R, kind="ExternalInput").ap()
    xkvT = nc.dram_tensor("xkvT", [D, S], F32R, kind="ExternalInput").ap()
    wq = nc.dram_tensor("wq", [D, HQ * P], F32R, kind="ExternalInput").ap()
    wk = nc.dram_tensor("wk", [D, P], F32R, kind="ExternalInput").ap()
    wv = nc.dram_tensor("wv", [D, P], F32R, kind="ExternalInput").ap()
    wo = nc.dram_tensor("wo", [HQ * P, D], F16, kind="ExternalInput").ap()
    perm = nc.dram_tensor("perm", [P, P], F32R, kind="ExternalInput").ap()
    cosq = nc.dram_tensor("cosq", [P, T], F16, kind="ExternalInput").ap()
    sinq = nc.dram_tensor("sinq", [P, T], F16, kind="ExternalInput").ap()
    cosk = nc.dram_tensor("cosk", [P, S], F16, kind="ExternalInput").ap()
    sink = nc.dram_tensor("sink", [P, S], F16, kind="ExternalInput").ap()
    out = nc.dram_tensor("out", [T, D], F16, kind="ExternalOutput").ap()

    xqT_t = xqT.rearrange("(dk p) t -> p dk t", p=P)
    xkvT_t = xkvT.rearrange("(dk p) t -> p dk t", p=P)
    wq_t = wq.rearrange("(dk p) h -> p dk h", p=P)
    wk_t = wk.rearrange("(dk p) h -> p dk h", p=P)
    wv_t = wv.rearrange("(dk p) h -> p dk h", p=P)
    wo_t = wo.rearrange("(h p) o -> p h o", p=P)

    with tile.TileContext(nc) as tc, ExitStack() as ctx:
        xp = ctx.enter_context(tc.tile_pool(name="xp", bufs=12))
        wp = ctx.enter_context(tc.tile_pool(name="wp", bufs=1))
        rp = ctx.enter_context(tc.tile_pool(name="rp", bufs=2))
        kvp = ctx.enter_context(tc.tile_pool(name="kvp", bufs=1))
        qp = ctx.enter_context(tc.tile_pool(name="qp", bufs=2))
        qp1 = ctx.enter_context(tc.tile_pool(name="qp1", bufs=1))
        qtp = ctx.enter_context(tc.tile_pool(name="qtp", bufs=6))
        ep = ctx.enter_context(tc.tile_pool(name="ep", bufs=3))
        op = ctx.enter_context(tc.tile_pool(name="op", bufs=2))
        outp = ctx.enter_context(tc.tile_pool(name="outp", bufs=4))
        sp = ctx.enter_context(tc.tile_pool(name="sp", bufs=2))
        ps_mm = ctx.enter_context(tc.tile_pool(name="ps_mm", bufs=2, space="PSUM"))
        ps_qp = ctx.enter_context(tc.tile_pool(name="ps_qp", bufs=2, space="PSUM"))
        ps_sc = ctx.enter_context(tc.tile_pool(name="ps_sc", bufs=3, space="PSUM"))
        ps_sum = ctx.enter_context(tc.tile_pool(name="ps_sum", bufs=1, space="PSUM"))

        DQ = 2  # dk-tiles per streamed x slab
        NSL = DK // DQ  # 4 slabs per chunk

        def alloc_slab(pfx, i):
            return xp.tile([P, DQ, CH], F32R, tag="x", name=f"{pfx}_{i}")

        def dma_slab(t, x_t, c, i):
            nc.sync.dma_start(t[:], x_t[:, bass.ts(i, DQ), bass.ts(c, CH)])

        # ---- DMA order: wk -> xkv0 -> wv -> tables -> wq ->
        #      (xq0/xkv1 interleaved) -> wo -> xq1; out DMAs at the end ----
        perm_sb = wp.tile([P, P], F32R)
        nc.sync.dma_start(perm_sb[:], perm)
        wk_sb = rp.tile([P, DK, P], F32R, tag="recycle", name="wk_sb")
        for i in range(4):
            nc.sync.dma_start(wk_sb[:, bass.ts(i, DK // 4), :],
                              wk_t[:, bass.ts(i, DK // 4), :])
        ident = wp.tile([P, P], BF16)
        make_identity(nc, ident[:])
        ones_sb = wp.tile([P, 1], BF16)
        nc.gpsimd.memset(ones_sb[:], 1.0)

        xkv0 = [alloc_slab("xk0", i) for i in range(NSL)]
        for i, t in enumerate(xkv0):
            dma_slab(t, xkvT_t, 0, i)
        wq_sb = wp.tile([P, DK, HQ * P], F32R)
        for i in range(4):
            nc.sync.dma_start(wq_sb[:, bass.ts(i, DK // 4), :],
                              wq_t[:, bass.ts(i, DK // 4), :])
        wv_sb = rp.tile([P, DK, P], F32R, tag="recycle", name="wv_sb")
        nc.sync.dma_start(wv_sb[:], wv_t)
        cosk_sb = wp.tile([P, S], F16)
        nc.sync.dma_start(cosk_sb[:], cosk)
        sink_sb = wp.tile([P, S], F16)
        nc.sync.dma_start(sink_sb[:], sink)
        cosq_sb = wp.tile([P, T], F16)
        nc.sync.dma_start(cosq_sb[:], cosq)
        sinq_sb = wp.tile([P, T], F16)
        nc.sync.dma_start(sinq_sb[:], sinq)
        # interleave the xq-chunk0 / xkv-chunk1 slab loads so the PE gets
        # both Q-projection and KV-projection work per DMA'd megabyte
        xq0, xkv1 = [], []
        for i in range(NSL):
            t = alloc_slab("xq0", i)
            dma_slab(t, xqT_t, 0, i)
            xq0.append(t)
            t = alloc_slab("xk1", i)
            dma_slab(t, xkvT_t, 1, i)
            xkv1.append(t)

        ktrot = kvp.tile([P, S], F32R)       # K^T after rope: [h', s]
        v_sb = kvp.tile([P, ST, P], BF16)   # V tiles: [s_in_tile, s_tile, h']

        def rope(psrc, cos_sb, sin_sb, dst, c, nm):
            """psrc: PSUM [P, CH] pre-rope [h',t]; writes rot into dst [P,CH].

            The half-swap q[(i+64)%128] comes from a PE permutation matmul so
            the DMA pipe stays free for HBM traffic.
            """
            q_sb = qp.tile([P, CH], F32R, tag="rope_in")
            nc.scalar.copy(q_sb[:], psrc[:])
            sw_ps = ps_sc.tile([P, CH], F32, tag="sc", name=f"swp_{nm}")
            nc.tensor.matmul(sw_ps[:], perm_sb[:], q_sb[:], start=True, stop=True)
            t1 = qp1.tile([P, CH], F32, tag="rope_t1")
            nc.vector.tensor_mul(t1[:], q_sb[:], cos_sb[:, bass.ts(c, CH)])
            t2 = qp1.tile([P, CH], F32, tag="rope_t2")
            nc.vector.tensor_mul(t2[:], sw_ps[:], sin_sb[:, bass.ts(c, CH)])
            nc.vector.tensor_add(dst, t1[:], t2[:])

        def proj(w_ap, slabs, pool, tag, name):
            ps = pool.tile([P, CH], F32, tag=tag, name=name)
            for dk in range(DK):
                nc.tensor.matmul(
                    ps[:], w_ap[:, dk, :], slabs[dk // DQ][:, dk % DQ, :],
                    start=(dk == 0), stop=(dk == DK - 1),
                )
            return ps

        def kv_chunk(c, xk):
            kps = proj(wk_sb, xk, ps_mm, "mm", f"kps{c}")
            rope(kps, cosk_sb, sink_sb, ktrot[:, bass.ts(c, CH)], c, f"k{c}")
            vps = proj(wv_sb, xk, ps_mm, "mm", f"vps{c}")
            vt_sb = qp1.tile([P, CH], BF16, tag="vt")
            nc.any.tensor_copy(vt_sb[:], vps[:])
            for i in range(CH // P):
                tps = ps_mm.tile([P, P], BF16, tag="mm", name=f"tps{c}_{i}")
                nc.tensor.transpose(tps[:], vt_sb[:, bass.ts(i, P)], ident[:])
                nc.vector.tensor_copy(v_sb[:, c * (CH // P) + i, :], tps[:])

        qtrot = {}
        exps_tiles = {}

        def q_chunk(c, xq):
            for h in range(HQ):
                qps = ps_qp.tile([P, CH], F32, tag="qp", name=f"qps{c}_{h}")
                for dk in range(DK):
                    nc.tensor.matmul(
                        qps[:], wq_sb[:, dk, bass.ts(h, P)],
                        xq[dk // DQ][:, dk % DQ, :],
                        start=(dk == 0), stop=(dk == DK - 1),
                    )
                qt = qtp.tile([P, CH], F32R, tag="qtrot", name=f"qtrot{c}_{h}")
                rope(qps, cosq_sb, sinq_sb, qt[:], c, f"q{c}_{h}")
                qtrot[(c, h)] = qt

        def scores_half(c, h, half):
            """Scores+exp+colsum for s-tiles in `half`; returns sums_sb."""
            if (c, h) not in exps_tiles:
                exps_tiles[(c, h)] = ep.tile(
                    [P, ST, CH], BF16, tag="exps", name=f"exps{c}_{h}")
            exps = exps_tiles[(c, h)]
            qt = qtrot[(c, h)]
            sums_ps = ps_sum.tile([1, CH], F32, tag="sum",
                                  name=f"sums{c}_{h}_{half}")
            # scores first (consecutive MMs), then the colsum accumulation
            # reusing the constant ones lhsT back-to-back
            sps_tiles = []
            for j in range(HST):
                st = half * HST + j
                sps = ps_sc.tile([P, CH], F32, tag="sc", name=f"sps{c}_{h}_{st}")
                nc.tensor.matmul(
                    sps[:], ktrot[:, bass.ts(st, P)], qt[:],
                    start=True, stop=True,
                )
                nc.scalar.activation(
                    exps[:, st, :], sps[:], mybir.ActivationFunctionType.Exp
                )
            for j in range(HST):
                st = half * HST + j
                nc.tensor.matmul(
                    sums_ps[:], ones_sb[:], exps[:, st, :],
                    start=(j == 0), stop=(j == HST - 1),
                )
            sums_sb = sp.tile([1, CH], F32, tag=f"sums{half}",
                              name=f"sumsb{c}_{h}_{half}")
            nc.vector.tensor_copy(sums_sb[:], sums_ps[:])
            return sums_sb

        def attn_finish(c, h, ot, sums_list):
            exps = exps_tiles.pop((c, h))
            if len(sums_list) > 1:
                tot = sp.tile([1, CH], F32, tag="sumtot", name=f"sumt{c}_{h}")
                nc.vector.tensor_add(tot[:], sums_list[0][:], sums_list[1][:])
            else:
                tot = sums_list[0]
            recip = sp.tile([1, CH], F32, tag="recip", name=f"recip{c}_{h}")
            nc.vector.reciprocal(recip[:], tot[:])
            rec_rep = qp.tile([P, CH], F32, tag="rec_rep")
            nc.gpsimd.partition_broadcast(rec_rep[:], recip[0:1, :])

            pv_ps = ps_mm.tile([P, CH], F32, tag="mm", name=f"pv{c}_{h}")
            for st in range(ST):
                nc.tensor.matmul(
                    pv_ps[:], v_sb[:, st, :], exps[:, st, :],
                    start=(st == 0), stop=(st == ST - 1),
                )
            nc.vector.tensor_mul(ot[:, h, :], pv_ps[:], rec_rep[:])

        def oproj(c, ot, wo_tiles):
            for tt in range(CH // P):
                for oc in range(D // CH):
                    ops_ = ps_mm.tile([P, CH], F32, tag="mm",
                                      name=f"op{c}_{tt}_{oc}")
                    for h in range(HQ):
                        nc.tensor.matmul(
                            ops_[:],
                            ot[:, h, bass.ts(tt, P)],
                            wo_tiles[h // 2][:, h % 2, bass.ts(oc, CH)],
                            start=(h == 0), stop=(h == HQ - 1),
                        )
                    o_out = outp.tile([P, CH], F16, tag="oout")
                    nc.any.tensor_copy(o_out[:], ops_[:])
                    nc.sync.dma_start(
                        out[c * CH + tt * P : c * CH + (tt + 1) * P,
                            bass.ts(oc, CH)],
                        o_out[:],
                    )

        # wo reuses the wk/wv SBUF slots (dead after the chunk-1
        # projections); loaded after xq0, before xq1
        wo_tiles = [rp.tile([P, 2, D], F16, tag="recycle", name=f"wo{i}")
                    for i in range(2)]
        for i in range(2):
            nc.sync.dma_start(wo_tiles[i][:], wo_t[:, bass.ts(i, 2), :])
        xq1 = [alloc_slab("xq1", i) for i in range(NSL)]
        for i, t in enumerate(xq1):
            dma_slab(t, xqT_t, 1, i)

        # ================= compute emission =================
        kv_chunk(0, xkv0)
        kv_chunk(1, xkv1)
        q_chunk(0, xq0)
        ot0 = op.tile([P, HQ, CH], F16, tag="ot", name="ot0")
        for h in range(HQ):
            sa = scores_half(0, h, 0)
            sb = scores_half(0, h, 1)
            attn_finish(0, h, ot0, [sa, sb])
        oproj(0, ot0, wo_tiles)
        q_chunk(1, xq1)
        ot1 = op.tile([P, HQ, CH], F16, tag="ot", name="ot1")
        for h in range(HQ):
            sa = scores_half(1, h, 0)
            sb = scores_half(1, h, 1)
            attn_finish(1, h, ot1, [sa, sb])
        oproj(1, ot1, wo_tiles)

    nc.compile()
    return nc


def _rope_tables(positions):
    """positions: (L,) int -> cos [128, L], sin_signed [128, L] fp16."""
    half = P // 2
    j = np.arange(half, dtype=np.float64)
    timescale = 10000.0 ** (2.0 * j / P)
    ang = positions.astype(np.float64)[None, :] / timescale[:, None]  # (64, L)
    cos = np.cos(ang)
    sin = np.sin(ang)
    cos_t = np.concatenate([cos, cos], axis=0).astype(np.float16)
    sin_t = np.concatenate([-sin, sin], axis=0).astype(np.float16)
    return np.ascontiguousarray(cos_t), np.ascontiguousarray(sin_t)


def kernel(Xq, Xkv, q_positions, kv_positions, Wq, Wk, Wv, Wo, _trace=False):
    Xq = np.asarray(Xq, dtype=np.float32)
    Xkv = np.asarray(Xkv, dtype=np.float32)
    q_positions = np.asarray(q_positions)
    kv_positions = np.asarray(kv_positions)
    Wq = np.asarray(Wq, dtype=np.float32)
    Wk = np.asarray(Wk, dtype=np.float32)
    Wv = np.asarray(Wv, dtype=np.float32)
    Wo = np.asarray(Wo, dtype=np.float32)

    B = Xq.shape[0]
    G = N_CORES // B  # kv-head groups per batch

    if "nc" not in _CACHE:
        _CACHE["nc"] = _build_program()
    nc = _CACHE["nc"]

    perm = np.ascontiguousarray(np.roll(np.eye(P, dtype=np.float32), P // 2, axis=0))

    per_b = {}
    for b in range(B):
        cos_q, sin_q = _rope_tables(q_positions[b])
        cos_k, sin_k = _rope_tables(kv_positions[b])
        per_b[b] = (np.ascontiguousarray(Xq[b].T), np.ascontiguousarray(Xkv[b].T),
                    cos_q, sin_q, cos_k, sin_k)
    in_maps = []
    for core in range(N_CORES):
        b, g = divmod(core, G)
        xqT_b, xkvT_b, cos_q, sin_q, cos_k, sin_k = per_b[b]
        in_maps.append({
            "xqT": xqT_b,
            "xkvT": xkvT_b,
            "wq": np.ascontiguousarray(
                Wq[:, g * HQ : (g + 1) * HQ, :].reshape(D, HQ * P)),
            "wk": np.ascontiguousarray(Wk[:, g, :]),
            "wv": np.ascontiguousarray(Wv[:, g, :]),
            "wo": np.ascontiguousarray(
                Wo[g * HQ : (g + 1) * HQ].reshape(HQ * P, D).astype(np.float16)),
            "perm": perm,
            "cosq": cos_q, "sinq": sin_q, "cosk": cos_k, "sink": sin_k,
        })

    r = run_bass_kernel_spmd(nc, in_maps, list(range(N_CORES)), trace=_trace)
    LAST_RUN["exec_time_ns"] = r.exec_time_ns
    LAST_RUN["mean_exec_time_ns"] = r.mean_exec_time_ns

    out = np.zeros((B, T, D), dtype=np.float32)
    for core in range(N_CORES):
        b = core // G
        out[b] += r.results[core]["out"].astype(np.float32)
    return out


# revision 33
# speedup vs baseline: 1.5586x; 1.0375x over previous
"""TRN2 Bass kernel for GQA attention (nn_Attention_13030930776201).

Reference computation (B=2, T=S=1024, D=2048, 16 Q heads / 4 KV heads, H=128):
    q = Xq @ Wq; k = Xkv @ Wk; v = Xkv @ Wv         (DenseGeneral projections)
    q, k = RoPE(q, q_pos), RoPE(k, kv_pos)
    out = softmax(q k^T) v  @ Wo                     (GQA, scale=1.0, no mask)

Sharding: 8 cores = 2 (batch) x 4 (KV-head group). Each core computes one
batch's attention for one KV head + its 4 Q heads, producing a partial
(1024, 2048) output; the host sums the 4 partials per batch.

Per-core dataflow (layouts chosen so NO on-device transposes of activations
are needed; the host passes X pre-transposed):
    QT[h',t]  = Wq[d,h'].T @ XqT[d,t]      (f32r matmuls, fp32 PSUM)
    KT[h',s]  = Wk[d,h'].T @ XkvT[d,s]
    VT[h',s]  = Wv[d,h'].T @ XkvT[d,s], PE-transposed to V[s,h'] (bf16)
    RoPE on QT/KT: half-swap via a PE permutation matmul + DVE muls (fp32)
    S^T[s,t]  = KT[h',s].T @ QT[h',t]      (f32r) ; expS = exp(S^T) (ACT, bf16)
    sums[t]   = ones[s].T @ expS[s,t]      (PE column-sum trick)
    O^T[h',t] = V[s,h'].T @ expS[s,t]      (bf16), normalized by 1/sums -> fp16
    out[t,o]  = O^T[h',t].T @ Wo[h',o]     (fp16), written as fp16 partial

float32r gives tf32-like precision at full PE rate (measured matmul rel err
1.5e-4 vs bf16's 2.4e-3), keeping softmax logits accurate; the V/O path is
precision-tolerant so it runs in bf16/fp16.

The emission order feeds the (serial, ~344 GB/s) DMA pipe just-in-time:
weights first, then X slabs in compute order; attention over chunk 0 is
split into s-halves so scores on the first KV chunk overlap the second
chunk's DMA + projections.
"""

import sys

if "/opt/trn_rl_repo" not in sys.path:
    sys.path.insert(0, "/opt/trn_rl_repo")

from contextlib import ExitStack

import numpy as np

import concourse.bass as bass
import concourse.tile as tile
from concourse import bacc, mybir
from concourse.bass_utils import run_bass_kernel_spmd
from concourse.masks import make_identity

P = 128          # partitions / head dim
T = 1024         # q tokens
S = 1024         # kv tokens
D = 2048         # model dim
DK = D // P      # 16 contraction tiles
CH = 512         # t/s chunk (psum free size)
NCH = T // CH    # 2
HQ = 4           # q heads per core
ST = S // P      # 8 s-tiles
HST = ST // 2    # s-tiles per half
N_CORES = 8

F32 = mybir.dt.float32
F32R = mybir.dt.float32r
BF16 = mybir.dt.bfloat16
F16 = mybir.dt.float16

_CACHE = {}
LAST_RUN = {}


def _build_program():
    nc = bacc.Bacc("TRN2", target_bir_lowering=False, debug=False, num_devices=1)

    xqT = nc.dram_tensor("xqT", [D, T], F32R, kind="ExternalInput").ap()
    xkvT = nc.dram_tensor("xkvT", [D, S], F32R, kind="ExternalInput").ap()
    wq = nc.dram_tensor("wq", [D, HQ * P], F32R, kind="ExternalInput").ap()
    wk = nc.dram_tensor("wk", [D, P], F32R, kind="ExternalInput").ap()
    wv = nc.dram_tensor("wv", [D, P], F32R, kind="ExternalInput").ap()
    wo = nc.dram_tensor("wo", [HQ * P, D], F16, kind="ExternalInput").ap()
    perm = nc.dram_tensor("perm", [P, P], F32R, kind="ExternalInput").ap()
    cosq = nc.dram_tensor("cosq", [P, T], F16, kind="ExternalInput").ap()
    sinq = nc.dram_tensor("sinq", [P, T], F16, kind="ExternalInput").ap()
    cosk = nc.dram_tensor("cosk", [P, S], F16, kind="ExternalInput").ap()
    sink = nc.dram_tensor("sink", [P, S], F16, kind="ExternalInput").ap()
    out = nc.dram_tensor("out", [T, D], F16, kind="ExternalOutput").ap()

    xqT_t = xqT.rearrange("(dk p) t -> p dk t", p=P)
    xkvT_t = xkvT.rearrange("(dk p) t -> p dk t", p=P)
    wq_t = wq.rearrange("(dk p) h -> p dk h", p=P)
    wk_t = wk.rearrange("(dk p) h -> p dk h", p=P)
    wv_t = wv.rearrange("(dk p) h -> p dk h", p=P)
    wo_t = wo.rearrange("(h p) o -> p h o", p=P)

    with tile.TileContext(nc) as tc, ExitStack() as ctx:
        xp = ctx.enter_context(tc.tile_pool(name="xp", bufs=12))
        wp = ctx.enter_context(tc.tile_pool(name="wp", bufs=1))
        rp = ctx.enter_context(tc.tile_pool(name="rp", bufs=2))
        kvp = ctx.enter_context(tc.tile_pool(name="kvp", bufs=1))
        qp = ctx.enter_context(tc.tile_pool(name="qp", bufs=2))
        qp1 = ctx.enter_context(tc.tile_pool(name="qp1", bufs=1))
        qtp = ctx.enter_context(tc.tile_pool(name="qtp", bufs=6))
        ep = ctx.enter_context(tc.tile_pool(name="ep", bufs=3))
        op = ctx.enter_context(tc.tile_pool(name="op", bufs=2))
        outp = ctx.enter_context(tc.tile_pool(name="outp", bufs=4))
        sp = ctx.enter_context(tc.tile_pool(name="sp", bufs=2))
        ps_mm = ctx.enter_context(tc.tile_pool(name="ps_mm", bufs=2, space="PSUM"))
        ps_qp = ctx.enter_context(tc.tile_pool(name="ps_qp", bufs=2, space="PSUM"))
        ps_sc = ctx.enter_context(tc.tile_pool(name="ps_sc", bufs=3, space="PSUM"))
        ps_sum = ctx.enter_context(tc.tile_pool(name="ps_sum", bufs=1, space="PSUM"))

        DQ = 2  # dk-tiles per streamed x slab
        NSL = DK // DQ  # 4 slabs per chunk

        def alloc_slab(pfx, i):
            return xp.tile([P, DQ, CH], F32R, tag="x", name=f"{pfx}_{i}")

        def dma_slab(t, x_t, c, i):
            nc.sync.dma_start(t[:], x_t[:, bass.ts(i, DQ), bass.ts(c, CH)])

        # ---- DMA order: wk -> xkv0 -> wv -> tables -> wq ->
        #      (xq0/xkv1 interleaved) -> wo -> xq1; out DMAs at the end ----
        wk_sb = rp.tile([P, DK, P], F32R, tag="recycle", name="wk_sb")
        xkv0 = [alloc_slab("xk0", i) for i in range(NSL)]
        # first K matmul needs only wk piece 0 + slab 0 -> load those first
        nc.sync.dma_start(wk_sb[:, bass.ts(0, DK // 4), :],
                          wk_t[:, bass.ts(0, DK // 4), :])
        dma_slab(xkv0[0], xkvT_t, 0, 0)
        dma_slab(xkv0[1], xkvT_t, 0, 1)
        for i in range(1, 4):
            nc.sync.dma_start(wk_sb[:, bass.ts(i, DK // 4), :],
                              wk_t[:, bass.ts(i, DK // 4), :])
        for i in range(2, NSL):
            dma_slab(xkv0[i], xkvT_t, 0, i)
        perm_sb = wp.tile([P, P], F32R)
        nc.sync.dma_start(perm_sb[:], perm)
        ident = wp.tile([P, P], BF16)
        make_identity(nc, ident[:])
        ones_sb = wp.tile([P, 1], BF16)
        nc.gpsimd.memset(ones_sb[:], 1.0)
        wq_sb = wp.tile([P, DK, HQ * P], F32R)
        for i in range(4):
            nc.sync.dma_start(wq_sb[:, bass.ts(i, DK // 4), :],
                              wq_t[:, bass.ts(i, DK // 4), :])
        wv_sb = rp.tile([P, DK, P], F32R, tag="recycle", name="wv_sb")
        nc.sync.dma_start(wv_sb[:], wv_t)
        cosk_sb = wp.tile([P, S], F16)
        nc.sync.dma_start(cosk_sb[:], cosk)
        sink_sb = wp.tile([P, S], F16)
        nc.sync.dma_start(sink_sb[:], sink)
        cosq_sb = wp.tile([P, T], F16)
        nc.sync.dma_start(cosq_sb[:], cosq)
        sinq_sb = wp.tile([P, T], F16)
        nc.sync.dma_start(sinq_sb[:], sinq)
        # interleave the xq-chunk0 / xkv-chunk1 slab loads so the PE gets
        # both Q-projection and KV-projection work per DMA'd megabyte
        xq0, xkv1 = [], []
        for i in range(NSL):
            t = alloc_slab("xq0", i)
            dma_slab(t, xqT_t, 0, i)
            xq0.append(t)
            t = alloc_slab("xk1", i)
            dma_slab(t, xkvT_t, 1, i)
            xkv1.append(t)

        ktrot = kvp.tile([P, S], F32R)       # K^T after rope: [h', s]
        v_sb = kvp.tile([P, ST, P], BF16)   # V tiles: [s_in_tile, s_tile, h']

        def rope(psrc, cos_sb, sin_sb, dst, c, nm):
            """psrc: PSUM [P, CH] pre-rope [h',t]; writes rot into dst [P,CH].

            The half-swap q[(i+64)%128] comes from a PE permutation matmul so
            the DMA pipe stays free for HBM traffic.
            """
            q_sb = qp.tile([P, CH], F32R, tag="rope_in")
            nc.scalar.copy(q_sb[:], psrc[:])
            sw_ps = ps_sc.tile([P, CH], F32, tag="sc", name=f"swp_{nm}")
            nc.tensor.matmul(sw_ps[:], perm_sb[:], q_sb[:], start=True, stop=True)
            t1 = qp1.tile([P, CH], F32, tag="rope_t1")
            nc.vector.tensor_mul(t1[:], q_sb[:], cos_sb[:, bass.ts(c, CH)])
            t2 = qp1.tile([P, CH], F32, tag="rope_t2")
            nc.vector.tensor_mul(t2[:], sw_ps[:], sin_sb[:, bass.ts(c, CH)])
            nc.vector.tensor_add(dst, t1[:], t2[:])

        def proj(w_ap, slabs, pool, tag, name):
            ps = pool.tile([P, CH], F32, tag=tag, name=name)
            for dk in range(DK):
                nc.tensor.matmul(
                    ps[:], w_ap[:, dk, :], slabs[dk // DQ][:, dk % DQ, :],
                    start=(dk == 0), stop=(dk == DK - 1),
                )
            return ps

        def kv_chunk(c, xk):
            kps = proj(wk_sb, xk, ps_mm, "mm", f"kps{c}")
            rope(kps, cosk_sb, sink_sb, ktrot[:, bass.ts(c, CH)], c, f"k{c}")
            vps = proj(wv_sb, xk, ps_mm, "mm", f"vps{c}")
            vt_sb = qp1.tile([P, CH], BF16, tag="vt")
            nc.any.tensor_copy(vt_sb[:], vps[:])
            for i in range(CH // P):
                tps = ps_mm.tile([P, P], BF16, tag="mm", name=f"tps{c}_{i}")
                nc.tensor.transpose(tps[:], vt_sb[:, bass.ts(i, P)], ident[:])
                nc.vector.tensor_copy(v_sb[:, c * (CH // P) + i, :], tps[:])

        qtrot = {}
        exps_tiles = {}

        def q_chunk(c, xq):
            for h in range(HQ):
                qps = ps_qp.tile([P, CH], F32, tag="qp", name=f"qps{c}_{h}")
                for dk in range(DK):
                    nc.tensor.matmul(
                        qps[:], wq_sb[:, dk, bass.ts(h, P)],
                        xq[dk // DQ][:, dk % DQ, :],
                        start=(dk == 0), stop=(dk == DK - 1),
                    )
                qt = qtp.tile([P, CH], F32R, tag="qtrot", name=f"qtrot{c}_{h}")
                rope(qps, cosq_sb, sinq_sb, qt[:], c, f"q{c}_{h}")
                qtrot[(c, h)] = qt

        def scores_half(c, h, half):
            """Scores+exp+colsum for s-tiles in `half`; returns sums_sb."""
            if (c, h) not in exps_tiles:
                exps_tiles[(c, h)] = ep.tile(
                    [P, ST, CH], BF16, tag="exps", name=f"exps{c}_{h}")
            exps = exps_tiles[(c, h)]
            qt = qtrot[(c, h)]
            sums_ps = ps_sum.tile([1, CH], F32, tag="sum",
                                  name=f"sums{c}_{h}_{half}")
            # scores first (consecutive MMs), then the colsum accumulation
            # reusing the constant ones lhsT back-to-back
            sps_tiles = []
            for j in range(HST):
                st = half * HST + j
                sps = ps_sc.tile([P, CH], F32, tag="sc", name=f"sps{c}_{h}_{st}")
                nc.tensor.matmul(
                    sps[:], ktrot[:, bass.ts(st, P)], qt[:],
                    start=True, stop=True,
                )
                nc.scalar.activation(
                    exps[:, st, :], sps[:], mybir.ActivationFunctionType.Exp
                )
            for j in range(HST):
                st = half * HST + j
                nc.tensor.matmul(
                    sums_ps[:], ones_sb[:], exps[:, st, :],
                    start=(j == 0), stop=(j == HST - 1),
                )
            sums_sb = sp.tile([1, CH], F32, tag=f"sums{half}",
                              name=f"sumsb{c}_{h}_{half}")
            nc.vector.tensor_copy(sums_sb[:], sums_ps[:])
            return sums_sb

        def attn_finish(c, h, ot, sums_list):
            exps = exps_tiles.pop((c, h))
            if len(sums_list) > 1:
                tot = sp.tile([1, CH], F32, tag="sumtot", name=f"sumt{c}_{h}")
                nc.vector.tensor_add(tot[:], sums_list[0][:], sums_list[1][:])
            else:
                tot = sums_list[0]
            recip = sp.tile([1, CH], F32, tag="recip", name=f"recip{c}_{h}")
            nc.vector.reciprocal(recip[:], tot[:])
            rec_rep = qp.tile([P, CH], F32, tag="rec_rep")
            nc.gpsimd.partition_broadcast(rec_rep[:], recip[0:1, :])

            pv_ps = ps_mm.tile([P, CH], F32, tag="mm", name=f"pv{c}_{h}")
            for st in range(ST):
                nc.tensor.matmul(
                    pv_ps[:], v_sb[:, st, :], exps[:, st, :],
                    start=(st == 0), stop=(st == ST - 1),
                )
            nc.vector.tensor_mul(ot[:, h, :], pv_ps[:], rec_rep[:])

        def oproj(c, ot, wo_tiles):
            for tt in range(CH // P):
                for oc in range(D // CH):
                    # alternate banks: the colsum bank is idle during oproj
                    if (tt * (D // CH) + oc) % 3 == 2:
                        ops_ = ps_sum.tile([P, CH], F32, tag="sum",
                                           name=f"op{c}_{tt}_{oc}")
                    else:
                        ops_ = ps_mm.tile([P, CH], F32, tag="mm",
                                          name=f"op{c}_{tt}_{oc}")
                    for h in range(HQ):
                        nc.tensor.matmul(
                            ops_[:],
                            ot[:, h, bass.ts(tt, P)],
                            wo_tiles[h // 2][:, h % 2, bass.ts(oc, CH)],
                            start=(h == 0), stop=(h == HQ - 1),
                        )
                    o_out = outp.tile([P, CH], F16, tag="oout")
                    nc.any.tensor_copy(o_out[:], ops_[:])
                    nc.sync.dma_start(
                        out[c * CH + tt * P : c * CH + (tt + 1) * P,
                            bass.ts(oc, CH)],
                        o_out[:],
                    )

        # wo reuses the wk/wv SBUF slots (dead after the chunk-1
        # projections); loaded after xq0, before xq1
        wo_tiles = [rp.tile([P, 2, D], F16, tag="recycle", name=f"wo{i}")
                    for i in range(2)]
        for i in range(2):
            nc.sync.dma_start(wo_tiles[i][:], wo_t[:, bass.ts(i, 2), :])
        xq1 = [alloc_slab("xq1", i) for i in range(NSL)]
        for i, t in enumerate(xq1):
            dma_slab(t, xqT_t, 1, i)

        # ================= compute emission =================
        kv_chunk(0, xkv0)
        kv_chunk(1, xkv1)
        q_chunk(0, xq0)
        ot0 = op.tile([P, HQ, CH], F16, tag="ot", name="ot0")
        for h in range(HQ):
            sa = scores_half(0, h, 0)
            sb = scores_half(0, h, 1)
            attn_finish(0, h, ot0, [sa, sb])
        oproj(0, ot0, wo_tiles)
        q_chunk(1, xq1)
        ot1 = op.tile([P, HQ, CH], F16, tag="ot", name="ot1")
        for h in range(HQ):
            sa = scores_half(1, h, 0)
            sb = scores_half(1, h, 1)
            attn_finish(1, h, ot1, [sa, sb])
        oproj(1, ot1, wo_tiles)

    nc.compile()
    return nc


def _rope_tables(positions):
    """positions: (L,) int -> cos [128, L], sin_signed [128, L] fp16."""
    half = P // 2
    j = np.arange(half, dtype=np.float64)
    timescale = 10000.0 ** (2.0 * j / P)
    ang = positions.astype(np.float64)[None, :] / timescale[:, None]  # (64, L)
    cos = np.cos(ang)
    sin = np.sin(ang)
    cos_t = np.concatenate([cos, cos], axis=0).astype(np.float16)
    sin_t = np.concatenate([-sin, sin], axis=0).astype(np.float16)
    return np.ascontiguousarray(cos_t), np.ascontiguousarray(sin_t)


def kernel(Xq, Xkv, q_positions, kv_positions, Wq, Wk, Wv, Wo, _trace=False):
    Xq = np.asarray(Xq, dtype=np.float32)
    Xkv = np.asarray(Xkv, dtype=np.float32)
    q_positions = np.asarray(q_positions)
    kv_positions = np.asarray(kv_positions)
    Wq = np.asarray(Wq, dtype=np.float32)
    Wk = np.asarray(Wk, dtype=np.float32)
    Wv = np.asarray(Wv, dtype=np.float32)
    Wo = np.asarray(Wo, dtype=np.float32)

    B = Xq.shape[0]
    G = N_CORES // B  # kv-head groups per batch

    if "nc" not in _CACHE:
        _CACHE["nc"] = _build_program()
    nc = _CACHE["nc"]

    perm = np.ascontiguousarray(np.roll(np.eye(P, dtype=np.float32), P // 2, axis=0))

    per_b = {}
    for b in range(B):
        cos_q, sin_q = _rope_tables(q_positions[b])
        cos_k, sin_k = _rope_tables(kv_positions[b])
        per_b[b] = (np.ascontiguousarray(Xq[b].T), np.ascontiguousarray(Xkv[b].T),
                    cos_q, sin_q, cos_k, sin_k)
    in_maps = []
    for core in range(N_CORES):
        b, g = divmod(core, G)
        xqT_b, xkvT_b, cos_q, sin_q, cos_k, sin_k = per_b[b]
        in_maps.append({
            "xqT": xqT_b,
            "xkvT": xkvT_b,
            "wq": np.ascontiguousarray(
                Wq[:, g * HQ : (g + 1) * HQ, :].reshape(D, HQ * P)),
            "wk": np.ascontiguousarray(Wk[:, g, :]),
            "wv": np.ascontiguousarray(Wv[:, g, :]),
            "wo": np.ascontiguousarray(
                Wo[g * HQ : (g + 1) * HQ].reshape(HQ * P, D).astype(np.float16)),
            "perm": perm,
            "cosq": cos_q, "sinq": sin_q, "cosk": cos_k, "sink": sin_k,
        })

    r = run_bass_kernel_spmd(nc, in_maps, list(range(N_CORES)), trace=_trace)
    LAST_RUN["exec_time_ns"] = r.exec_time_ns
    LAST_RUN["mean_exec_time_ns"] = r.mean_exec_time_ns

    out = np.zeros((B, T, D), dtype=np.float32)
    for core in range(N_CORES):
        b = core // G
        out[b] += r.results[core]["out"].astype(np.float32)
    return out
